# revision 2
# baseline (speedup 1.0000x reference)
"""Trainium2 Bass kernel v2 for nn_Dilation2D (morphological dilation).

    out[b,y,x,c] = max_{i,j} ( x_pad[b, y+i-1, x+j-1, c] + w[i,j,c] )

Sharding: pure data parallel over batch B=8 -> one image per NeuronCore.

Per-core layout (as baseline): partitions p = hb*32 + c, free = (row, x),
fp16 tap pipeline, PE-transpose relayout in/out, ACT PSUM<->SBUF copies.

v2 tap engine: instead of 16 fused 2x taps (0.5 cyc/elem each = 8 cyc/elem),
each filter row i runs ONE hand-written 8-slice 1x DVE pass (TAPQUAD) that
computes all 4 horizontal taps at 1 cyc/elem:

    Q_i[y,x] = max_{j=0..3}( x[y+i-1, x+j-1] + w[i,j] )

using the stream delay tricks: Src0 = x window, Src1 = same window offset -2,
a swap-flop delay slice for x[e-1], and slice-1's a-flop read via
NEXT_ALU_OUT_A (value from stream step e-2) for x[e-3]. w[i,3]/w[i,2] ride
the C0/C1 per-partition scalars; w[i,0]/w[i,1] are pre-latched into slice
0/3 swap flops by a tiny 2-element TAPLATCH instruction emitted immediately
before each TAPQUAD (same-engine program order; swap flops persist across
instruction boundaries). The 3 merge maxes max(Q0..Q3) run on GPSIMD/DVE.
"""

import numpy as np

import concourse.bass as bass
import concourse.bacc as bacc
import concourse.tile as tile
import concourse.dve_ops as dve_ops
from concourse import mybir
from concourse.bass_utils import run_bass_kernel_spmd
from concourse.instruction_name_ordered_set import InstructionNameOrderedSet
from concourse.dve_spec import Spec, Src0, Src1, C0, C1, maxx, Leaf
from concourse.dve_uop import (
    DveOpSpec, UopConfig, UopDpConfig, InpSel, OutSel, OutPath, AluOp,
    AluInp, DelayInp, Trigger,
)

# Problem constants (hardcoded per contract).
B, H, W, C = 8, 512, 512, 32
KH, KW = 4, 4
HBLK = 4               # row blocks on partitions
HB = H // HBLK         # 128 rows per block
XC = W // 128          # 4 x-chunks of 128 pixels
YT = 8                 # output rows per chunk
NCHUNK = HB // YT      # 16 chunks
RT = YT + KH - 1       # 11 input rows per chunk (with halo)
XOFF = 16              # x=0 lives at column 16
XW = 544               # padded row width
NEG = -60000.0         # -inf stand-in that fits fp16
LEAD = 8               # quad-pass lead-in elements (flush delay flops)
QW = 528               # Q row pitch (elements); valid x at cols [LEAD, LEAD+W)

F32 = mybir.dt.float32
F16 = mybir.dt.float16
AX = mybir.AluOpType

_ENABLE, _DISABLE = 1, 0

# Module-level side channel so the (unused-at-runtime) numpy references of
# TAPLATCH/TAPQUAD stay self-consistent when a bass-level interpreter runs
# them in program order. b16 executes the uop tables and never calls these.
_LATCH_STATE = {}


def _dp(op=AluOp.BYPASS, a=AluInp.PREV_ALU_OUT, b=None, *, swap=False,
        aflop=False, pass_lanes=(), caps=()):
    d = UopDpConfig()
    d.enable_alu(op, a, b if b is not None else a)
    if swap:
        d.swap_enable = _ENABLE
    if aflop:
        d.alu_out_a_enable = _ENABLE
    d.pass_through_delay(*pass_lanes)
    for src, ln in caps:
        d.enable_delay_from_src(src, ln)
    return d


PD = AluInp  # shorthand


def _build_quad_uop():
    """out[e] = max(Src0[e]+C0, Src0[e-1]+swap@s2, Src1[e]+swap@s3,
                    Src0[e-3]+swap@s0)

    Lanes: 0 = Src0 (x0), 1 = Src1 (x2), 2 = C0 (w[i,3]),
    3 = A2 capture, 4 = A3 capture, 5 = A1 capture.
    """
    u = UopConfig()
    for ln, sel in ((1, InpSel.SRC_0), (2, InpSel.SRC_1),
                    (3, InpSel.CONST_0)):
        u.inp[ln] = sel
        u.inp_enable[ln] = _ENABLE
    CAP = DelayInp.PREV_ALU_OUT
    u.datapath_config = [
        # s0: A3 = x[e-3] (s1's a-flop, 2 steps back) + w[i,0] (swap@s0)
        _dp(AluOp.ADD, PD.NEXT_ALU_OUT_A, PD.CURR_SWAP_OUT,
            pass_lanes=(0, 1, 2)),
        # s1: delay slice: emits x[e-1]; swap <- x0[e]; a-flop feeds s0.
        _dp(AluOp.BYPASS, PD.CURR_SWAP_OUT, PD.PREV_DELAY_0,
            swap=True, aflop=True, pass_lanes=(0, 1, 2),
            caps=((CAP, 4),)),
        # s2: A1 = x[e-1] + w[i,2] (swap@s2)
        _dp(AluOp.ADD, PD.PREV_ALU_OUT, PD.CURR_SWAP_OUT,
            pass_lanes=(0, 1, 2, 4)),
        # s3: A2 = x[e-2] (Src1, lane 1) + w[i,1] (swap@s3); capture A1 -> 5
        _dp(AluOp.ADD, PD.PREV_DELAY_1, PD.CURR_SWAP_OUT,
            pass_lanes=(0, 2, 4), caps=((CAP, 5),)),
        # s4: A0 = x0 + w[i,3] (C0 on lane 2); capture A2 -> lane 3
        _dp(AluOp.ADD, PD.PREV_DELAY_0, PD.PREV_DELAY_2,
            pass_lanes=(4, 5), caps=((CAP, 3),)),
        # s5: M1 = max(A0, A1)
        _dp(AluOp.MAX, PD.PREV_ALU_OUT, PD.PREV_DELAY_5,
            pass_lanes=(3, 4)),
        # s6: M2 = max(M1, A2)
        _dp(AluOp.MAX, PD.PREV_ALU_OUT, PD.PREV_DELAY_3,
            pass_lanes=(4,)),
        # s7: M3 = max(M2, A3)
        _dp(AluOp.MAX, PD.PREV_ALU_OUT, PD.PREV_DELAY_4),
    ]
    u.out = {OutPath.WR0_LO: OutSel.ALU_OUT, OutPath.WR0_HI: OutSel.ALU_OUT,
             OutPath.WR1_LO: OutSel.ALU_OUT, OutPath.WR1_HI: OutSel.ALU_OUT}
    u.out_enable = {OutPath.WR0_LO: _ENABLE, OutPath.WR0_HI: _DISABLE,
                    OutPath.WR1_LO: _DISABLE, OutPath.WR1_HI: _DISABLE}
    u.require_inp0 = _ENABLE
    u.require_inp1 = _ENABLE
    u.trigger = (Trigger.SRC_TENSOR_DONE, Trigger.NONE, Trigger.NONE)
    u.next_uop = (0, 0, 0)
    u.repeat_count = 0
    return u


def _build_latch_uops():
    """3-element op: elem0 -> swap@s0 (uop0), elem1 -> swap@s3 (uop1),
    elem2 -> swap@s2 (uop2). Only stage 0 / the swap stage differ per uop;
    each element sees exactly one uop's config end-to-end."""
    chain = lambda: _dp(AluOp.BYPASS, PD.PREV_ALU_OUT, PD.PREV_ALU_OUT)

    def mk(swap_stage, trig, nxt, rpt):
        u = UopConfig()
        u.inp[1] = InpSel.SRC_0
        u.inp_enable[1] = _ENABLE
        dp = []
        for s in range(8):
            if s == 0:
                if swap_stage == 0:
                    dp.append(_dp(AluOp.BYPASS, PD.PREV_DELAY_0,
                                  PD.PREV_DELAY_0, swap=True))
                else:
                    dp.append(_dp(AluOp.BYPASS, PD.PREV_DELAY_0,
                                  PD.PREV_DELAY_0))
            elif s == swap_stage:
                dp.append(_dp(AluOp.BYPASS, PD.PREV_ALU_OUT,
                              PD.PREV_ALU_OUT, swap=True))
            else:
                dp.append(chain())
        u.datapath_config = dp
        u.out = {OutPath.WR0_LO: OutSel.ALU_OUT,
                 OutPath.WR0_HI: OutSel.ALU_OUT,
                 OutPath.WR1_LO: OutSel.ALU_OUT,
                 OutPath.WR1_HI: OutSel.ALU_OUT}
        u.out_enable = {OutPath.WR0_LO: _ENABLE, OutPath.WR0_HI: _DISABLE,
                        OutPath.WR1_LO: _DISABLE, OutPath.WR1_HI: _DISABLE}
        u.require_inp0 = _ENABLE
        u.require_inp1 = _DISABLE
        u.trigger = (trig, Trigger.NONE, Trigger.NONE)
        u.next_uop = (nxt, 0, 0)
        u.repeat_count = rpt
        return u

    return [
        mk(0, Trigger.COUNT, 1, 1),
        mk(3, Trigger.COUNT, 2, 1),
        mk(2, Trigger.SRC_TENSOR_DONE, 0, 0),
    ]


def _shift_stream(a, k, fill):
    """Shift right by k along the flattened free stream (per partition)."""
    P = a.shape[0]
    flat = a.reshape(P, -1)
    out = np.full_like(flat, fill)
    if k == 0:
        out[:] = flat
    else:
        out[:, k:] = flat[:, :-k]
    return out.reshape(a.shape)


def _register_ops():
    names = {o.name: o for o in dve_ops.OPS}
    made = []
    if "TAPLATCH_ANT" in names and "TAPQUAD_ANT" in names:
        return names["TAPLATCH_ANT"], names["TAPQUAD_ANT"]

    def _latch_ref(in0, in1, s0, s1, imm2):
        _LATCH_STATE["pair"] = np.asarray(in0, dtype=np.float32).copy()
        return np.asarray(in0, dtype=np.float32)

    latch_spec = Spec(body=Src0 + Leaf(InpSel.ZERO), reference=_latch_ref)

    def _quad_ref(in0, in1, s0, s1, imm2):
        in0 = np.asarray(in0, dtype=np.float32)
        in1 = np.asarray(in1, dtype=np.float32).reshape(in0.shape)
        s0 = np.asarray(s0, dtype=np.float32).reshape(in0.shape[0], *([1] * (in0.ndim - 1)))
        pair = _LATCH_STATE.get("pair")
        w0 = pair[:, 0].reshape(s0.shape)
        w1 = pair[:, 1].reshape(s0.shape)
        w2 = pair[:, 2].reshape(s0.shape)
        t0 = in0 + s0
        t1 = _shift_stream(in0, 1, -1e30) + w2
        t2 = in1 + w1
        t3 = _shift_stream(in0, 3, -1e30) + w0
        return np.maximum(np.maximum(t0, t1), np.maximum(t2, t3))

    quad_spec = Spec(body=maxx(maxx(Src0 + C0, Src0), Src1),
                     reference=_quad_ref)

    row0 = dve_ops._CUSTOM_DVE_ROW_BASE + len(dve_ops.OPS)
    latch_full = DveOpSpec(name="TAPLATCH_ANT", opcode=row0,
                           uops=_build_latch_uops(), uops_2x=None,
                           rd1_en=False, perf_max=0)
    for u in latch_full.uops:
        u.validate("v3")
    latch_full.validate("v3")
    latch_op = dve_ops.DveOp("TAPLATCH_ANT", latch_spec, subdim=False,
                             uops_sha={"v3": latch_full.sha("v3")})
    dve_ops.OPS.append(latch_op)
    dve_ops._SUB_OPCODE_FOR_NAME["TAPLATCH_ANT"] = row0
    dve_ops.CUSTOM_DVE_SPECS["TAPLATCH_ANT"] = latch_spec
    dve_ops._COMPILE_CACHE[("TAPLATCH_ANT", "v3")] = latch_full

    row1 = dve_ops._CUSTOM_DVE_ROW_BASE + len(dve_ops.OPS)
    qu = _build_quad_uop()
    qu.validate("v3")
    quad_full = DveOpSpec(name="TAPQUAD_ANT", opcode=row1, uops=[qu],
                          uops_2x=None, rd1_en=True, perf_max=0)
    quad_full.validate("v3")
    quad_op = dve_ops.DveOp("TAPQUAD_ANT", quad_spec, subdim=False,
                            uops_sha={"v3": quad_full.sha("v3")})
    dve_ops.OPS.append(quad_op)
    dve_ops._SUB_OPCODE_FOR_NAME["TAPQUAD_ANT"] = row1
    dve_ops.CUSTOM_DVE_SPECS["TAPQUAD_ANT"] = quad_spec
    dve_ops._COMPILE_CACHE[("TAPQUAD_ANT", "v3")] = quad_full
    return latch_op, quad_op


def _build_program(repeat: int = 1):
    latch_op, quad_op = _register_ops()
    nc = bacc.Bacc("TRN2", target_bir_lowering=False, debug=False)
    x_d = nc.dram_tensor("x", [H, W, C], F32, kind="ExternalInput").ap()
    w_d = nc.dram_tensor("w", [KH, KW, C], F32, kind="ExternalInput").ap()
    o_d = nc.dram_tensor("out", [H, W, C], F32, kind="ExternalOutput").ap()

    x_v = x_d.rearrange("(hb y) (xc q) c -> q y xc hb c", hb=HBLK, q=128)
    o_v = o_d.rearrange("(hb y) (xc q) c -> q y xc hb c", hb=HBLK, q=128)

    with tile.TileContext(nc) as tc:
        consts = tc.alloc_tile_pool(name="consts", bufs=1)
        w_sb = consts.tile([128, KH * KW], F32)
        w_r = w_d.rearrange("i j c -> c (i j)")
        for hb in range(HBLK):
            nc.sync.dma_start(out=w_sb[32 * hb : 32 * (hb + 1), :], in_=w_r)
        import concourse.masks as masks
        id32 = consts.tile([128, 128], F32)
        masks.make_identity(nc, id32[:])
        id16 = consts.tile([128, 128], F16)
        masks.make_identity(nc, id16[:])
        wdummy = consts.tile([128, 3], F32, tag="wdummy")

        pre32_pool = tc.alloc_tile_pool(name="pre32", bufs=2)
        xbuf_pool = tc.alloc_tile_pool(name="xbuf", bufs=2)
        q_pool = tc.alloc_tile_pool(name="qbuf", bufs=2)
        ost_pool = tc.alloc_tile_pool(name="ost", bufs=2)
        psi_pool = tc.alloc_tile_pool(name="psi", bufs=3, space="PSUM")
        pso_pool = tc.alloc_tile_pool(name="pso", bufs=3, space="PSUM")

        # The TAPLATCH -> TAPQUAD pairing relies on swap-flop state carried
        # between instructions, so the DVE stream must execute in emission
        # order. Tile may otherwise reorder; chain every DVE instruction to
        # its predecessor with a nosync dep (free on an in-order engine).
        _prev_dve = [None]

        def dve_chain(cur):
            prev = _prev_dve[0]
            if prev is not None:
                s = InstructionNameOrderedSet()
                s.add(prev.ins.name)
                cur.ins.add_nosync_dependencies_from(s)
            _prev_dve[0] = cur
            return cur

        prev_xt_v = [None]

        for ck_rep in range(NCHUNK * repeat):
            ck = ck_rep % NCHUNK
            y0 = ck * YT  # first output row (within each hb block)

            # ---- load: DRAM -> pre32 [q, (r, xc, hb, c)]. Chunk 0 loads the
            # full RT=11 rows; later chunks copy the 3 overlap rows from the
            # previous x-tile (ACT, fp16) and load only YT=8 new rows. ----
            pre32 = pre32_pool.tile([128, RT * XC * HBLK * C], F32)
            p32v = pre32[:].rearrange(
                "q (r xc hb c) -> q r xc hb c", r=RT, xc=XC, hb=HBLK, c=C
            )
            xt = xbuf_pool.tile([128, RT * XW], F16)
            xt_v = xt[:].rearrange("p (r x) -> p r x", r=RT, x=XW)

            if ck == 0:
                nload, rt_base = RT, 0
                for hb in range(HBLK):
                    nc.sync.dma_start(
                        out=p32v[:, 1:RT, :, hb],
                        in_=x_v[:, y0 : y0 + RT - 1, :, hb],
                    )
                    nc.sync.dma_start(
                        out=p32v[:, 0, :, hb],
                        in_=x_v[:, HB - 1 if hb else 0, :, max(hb - 1, 0)],
                    )
            else:
                nload, rt_base = YT, 3
                nc.scalar.copy(xt_v[:, 0:3, :], prev_xt_v[0][:, RT - 3 : RT, :])
                for hb in range(HBLK):
                    if ck == NCHUNK - 1:
                        nc.sync.dma_start(
                            out=p32v[:, 0:6, :, hb],
                            in_=x_v[:, y0 + 2 : y0 + 8, :, hb],
                        )
                        nc.sync.dma_start(
                            out=p32v[:, 6:8, :, hb],
                            in_=x_v[:, 0:2, :, min(hb + 1, HBLK - 1)],
                        )
                    else:
                        nc.sync.dma_start(
                            out=p32v[:, 0:YT, :, hb],
                            in_=x_v[:, y0 + 2 : y0 + 2 + YT, :, hb],
                        )

            # ---- relayout: PE transpose [q,(hb,c)] -> [(hb,c),q] into PSUM;
            # ACT copies PSUM -> x-tile, casting f32 -> fp16 ----
            for r in range(nload):
                ps = psi_pool.tile([128, XC * 128], F32)
                for xc in range(XC):
                    nc.tensor.matmul(
                        ps[:, 128 * xc : 128 * (xc + 1)],
                        p32v[:, r, xc],
                        id32[:],
                        start=(xc == 0),
                        stop=(xc == XC - 1),
                        is_transpose=True,
                        skip_group_check=True,
                    )
                nc.scalar.copy(xt_v[:, rt_base + r, XOFF : XOFF + W], ps[:])
            prev_xt_v[0] = xt_v

            # borders: pads memset once per rotating buffer (first 2 chunks);
            # loads never touch them and the 3-row copy carries them along.
            if ck_rep % NCHUNK < 2:
                nc.gpsimd.memset(xt_v[:, :, XOFF - 4 : XOFF], NEG)
                nc.gpsimd.memset(xt_v[:, :, XOFF + W :], NEG)
            if ck == 0:
                nc.gpsimd.memset(xt_v[0:32, 0, :], NEG)          # hb=0, y=-1
            if ck == NCHUNK - 1:
                nc.gpsimd.memset(xt_v[96:128, RT - 2 : RT, :], NEG)  # hb=3

            # ---- taps: per filter row i, latch w[i,0:2] then one TAPQUAD
            # pass producing Q_i; merge Q's (2 on gpsimd, 1 on DVE) ----
            qs = [
                q_pool.tile([128, YT * QW], F16, tag=f"q{k}", name=f"q{k}")
                for k in range(KH)
            ]
            qvs = [
                q.rearrange("p (r x) -> p r x", r=YT, x=QW) for q in
                (qt[:] for qt in qs)
            ]
            # Streams run RIGHT-TO-LEFT (descending x): a pipeline bubble at
            # instruction start can make NEXT_ALU_OUT_A transiently one
            # element too recent; in descending order the affected boundary
            # position then reads the right NEG pad instead of real data.
            # Mapping (out x, stream step t): x = W-1-(t-LEAD);
            #   A0 = Src0[t]   = col x-1 -> + w[i,0] (C0 scalar)
            #   A1 = Src0[t-1] = col x   -> + w[i,1] (swap@s2)
            #   A2 = Src1[t]   = col x+1 -> + w[i,2] (swap@s3)
            #   A3 = Src0[t-3] = col x+2 -> + w[i,3] (swap@s0)
            SL = W + LEAD  # stream length per row
            C0X = XOFF + W - 2 + LEAD   # src0 start col (t=0)
            QTOP = 2 * LEAD + W - 1     # out start col (t=0)
            for i in range(KH):
                dve_chain(nc.vector._custom_dve(
                    latch_op, out=wdummy[:],
                    in0=w_sb[:, 4 * i + 3 : 4 * i : -1],
                ))
                src0 = xt_v[:, i : i + YT, C0X : C0X - SL : -1]
                src1 = xt_v[:, i : i + YT, C0X + 2 : C0X + 2 - SL : -1]
                dve_chain(nc.vector._custom_dve(
                    quad_op, out=qvs[i][:, :, QTOP : QTOP - SL : -1],
                    in0=src0, in1=src1,
                    s0=w_sb[:, 4 * i : 4 * i + 1],
                ))

            def qval(k):
                return qvs[k][:, :, LEAD : LEAD + W]

            dve_chain(nc.vector.tensor_tensor(out=qval(1), in0=qval(1),
                                              in1=qval(0), op=AX.max))
            dve_chain(nc.vector.tensor_tensor(out=qval(3), in0=qval(3),
                                              in1=qval(2), op=AX.max))
            dve_chain(nc.vector.tensor_tensor(out=qval(3), in0=qval(3),
                                              in1=qval(1), op=AX.max))

            # ---- transpose back on PE (fp16), ACT copy casts to f32, store
            ost32 = ost_pool.tile([128, YT * XC * HBLK * C], F32, tag="o32")
            o32v = ost32[:].rearrange(
                "q (r xc hb c) -> q r xc hb c", r=YT, xc=XC, hb=HBLK, c=C
            )
            qf = qvs[3]
            for r in range(YT):
                ps = pso_pool.tile([128, XC * 128], F16)
                for xc in range(XC):
                    nc.tensor.matmul(
                        ps[:, 128 * xc : 128 * (xc + 1)],
                        qf[:, r, LEAD + 128 * xc : LEAD + 128 * (xc + 1)],
                        id16[:],
                        start=(xc == 0),
                        stop=(xc == XC - 1),
                        is_transpose=True,
                        skip_group_check=True,
                    )
                nc.scalar.copy(o32v[:, r], ps[:])
            for hb in range(HBLK):
                nc.sync.dma_start(
                    out=o_v[:, y0 : y0 + YT, :, hb], in_=o32v[:, :, :, hb]
                )

        for p in (pso_pool, psi_pool, ost_pool, q_pool,
                  xbuf_pool, pre32_pool, consts):
            p.release()

    nc.compile()
    return nc


_CACHED = {}


def _get_program(repeat: int = 1):
    if repeat not in _CACHED:
        _CACHED[repeat] = _build_program(repeat)
    return _CACHED[repeat]


def kernel(x: np.ndarray, w: np.ndarray, _trace: bool = False,
           _repeat: int = 1):
    """Full inputs in, full output out. Shards batch across 8 cores."""
    x = np.ascontiguousarray(np.asarray(x), dtype=np.float32)
    w = np.ascontiguousarray(np.asarray(w), dtype=np.float32)
    assert x.shape == (B, H, W, C) and w.shape == (KH, KW, C)
    nc = _get_program(_repeat)
    core_ids = list(range(B))
    in_maps = [{"x": x[b], "w": w} for b in range(B)]
    res = run_bass_kernel_spmd(nc, in_maps, core_ids, trace=_trace)
    out = np.stack([res.results[i]["out"] for i in range(B)], axis=0)
    if _trace:
        kernel.last_exec_time_ns = res.exec_time_ns
        kernel.last_results = res
    return out


if __name__ == "__main__":
    rng = np.random.default_rng(0)
    x = rng.standard_normal((B, H, W, C), dtype=np.float32)
    w = (rng.standard_normal((KH, KW, C)) * 0.1).astype(np.float32)
    out = kernel(x, w)

    # numpy oracle
    xp = np.full((B, H + 3, W + 3, C), -np.inf, dtype=np.float32)
    xp[:, 1:1 + H, 1:1 + W, :] = x
    exp = np.full_like(x, -np.inf)
    for i in range(4):
        for j in range(4):
            exp = np.maximum(exp, xp[:, i:i + H, j:j + W, :] + w[i, j])
    err = np.abs(out - exp)
    denom = max(float(np.abs(exp).max()), 1e-12)
    print("abs max err:", float(err.max()))
    print("Relative error:", float(err.max()) / denom)
